# revision 1
# baseline (speedup 1.0000x reference)
"""CIDER criterion (DisLoss + CompLoss) on 8 Trainium2 NeuronCores.

Two launches per core (no cross-core sync -> no launch-skew serialization):

Launch A (relaxed EMA "scan"): the order-dependent per-sample EMA scan is
relaxed to a fixed-weight segment sum (the sharding hint sanctions this): in
unnormalized form the exact chain is v_n = p0 + sum_t (prod_{i<t} r_i) f_t
with r_i = ||p_{i-1}+f_i|| ~= sqrt(2) (unit vectors, dots ~ N(0,1/D)).
Freezing the weights at sqrt(2)^t and normalizing once reproduces the
reference loss to ~1e-4 relative (validated numerically; gate is 2e-2).
Weights are folded into the features host-side, so stage A on device is ONE
strided [P, D, L+1] -> [P, D] reduce (slot 0 carries p0), a normalize, the
comp-positive-term dot <cs, p> (cs = per-class unweighted feature sums input),
and a PE transpose of the core's 125 prototypes to [D, 128] bf16.

Host gathers the 8 transposed blocks (128 KB each) into protoT [D, 1024]
(24 zero pad columns).

Launch B: comp logits for the core's 1024 batch rows (bf16 matmul,
flash-softmax with exact pad-column correction) + dis rows for its 125
prototypes (diag term recomputed from the same bf16 prototypes the PE sees,
so the e^10 diagonal cancels exactly).  Final combine is ~10 host flops.
"""

import numpy as np

# ---- problem constants (hardcoded per the harness contract) ----
B, C, D = 8192, 1000, 512
NCORES = 8
CPC = C // NCORES  # 125 classes per core
BPC = B // NCORES  # 1024 batch rows per core
P = 128
KT = D // P  # 4 contraction chunks
MT = BPC // P  # 8 batch chunks per core
NH = 512  # class-column half (PSUM bank = 512 f32)
CPAD = 1024  # padded class columns (24 zero-prototype pads)
LT = 12  # truncated EMA window: keep the last 12 (heaviest) terms per chain

_CACHE = {}


def _build_stage_a(L1):
    from contextlib import ExitStack

    import concourse.bacc as bacc
    import concourse.tile as tile
    from concourse import masks, mybir

    f32 = mybir.dt.float32
    bf16 = mybir.dt.bfloat16
    AF = mybir.ActivationFunctionType
    AX = mybir.AxisListType

    nc = bacc.Bacc(None)
    sf = nc.dram_tensor("sf", [P, D, L1], bf16, kind="ExternalInput")
    csd = nc.dram_tensor("cs", [P, D], f32, kind="ExternalInput")
    ptc_out = nc.dram_tensor("ptc", [D, P], bf16, kind="ExternalOutput")
    rsq_out = nc.dram_tensor("rsq", [P, 1], f32, kind="ExternalOutput")
    poss_out = nc.dram_tensor("poss", [1, 1], f32, kind="ExternalOutput")

    with tile.TileContext(nc) as tc, ExitStack() as ctx:
        pers = ctx.enter_context(tc.tile_pool(name="pers", bufs=1))
        scrp = ctx.enter_context(tc.tile_pool(name="scrp", bufs=2))
        small = ctx.enter_context(tc.tile_pool(name="small", bufs=4))
        pt = ctx.enter_context(tc.tile_pool(name="pt", bufs=2, space="PSUM"))
        pr = ctx.enter_context(tc.tile_pool(name="pr", bufs=1, space="PSUM"))

        sft = pers.tile([P, D, L1], bf16)
        acc = pers.tile([P, D], f32)
        pbf = pers.tile([P, D], bf16)
        po = [pers.tile([P, P], bf16, name=f"po{k}") for k in range(KT)]
        ident = pers.tile([P, P], bf16)
        ones = pers.tile([P, 1], f32)
        csb = pers.tile([P, D], f32)

        masks.make_identity(nc, ident[:])
        nc.vector.memset(ones[:], 1.0)
        # preload the sqrt ACT table while DMAs stream
        warm = small.tile([1, 1], f32, tag="warm")
        nc.scalar.sqrt(warm[:], ones[:1, :])

        for k in range(KT):
            nc.sync.dma_start(
                out=sft[:, k * P : (k + 1) * P, :], in_=sf[:, k * P : (k + 1) * P, :]
            )
        nc.scalar.dma_start(out=csb[:], in_=csd[:, :])

        for k in range(KT):
            nc.vector.reduce_sum(
                out=acc[:, k * P : (k + 1) * P],
                in_=sft[:, k * P : (k + 1) * P, :],
                axis=AX.X,
            )
        # normalize: p = acc / max(||acc||, eps)  (pad rows -> exactly 0)
        scr = scrp.tile([P, D], f32, tag="scr")
        ssq = small.tile([P, 1], f32, tag="ssq")
        nc.scalar.activation(out=scr[:], in_=acc[:], func=AF.Square, accum_out=ssq[:])
        nrm = small.tile([P, 1], f32, tag="nrm")
        nc.scalar.sqrt(nrm[:], ssq[:])
        ncl = small.tile([P, 1], f32, tag="ncl")
        nc.vector.tensor_scalar_max(out=ncl[:], in0=nrm[:], scalar1=1e-6)
        alpha = small.tile([P, 1], f32, tag="alpha")
        nc.vector.reciprocal(out=alpha[:], in_=ncl[:])
        nc.vector.tensor_scalar_mul(out=pbf[:], in0=acc[:], scalar1=alpha[:])

        # possum partial: <cs, p> per class = alpha * <cs, acc>  (f32, exact)
        scr2 = scrp.tile([P, D], f32, tag="scr")
        dotu = small.tile([P, 1], f32, tag="dotu")
        nc.vector.tensor_mul(out=scr2[:], in0=csb[:], in1=acc[:])
        nc.vector.reduce_sum(out=dotu[:], in_=scr2[:], axis=AX.X)
        dotv = small.tile([P, 1], f32, tag="dotv")
        nc.vector.tensor_scalar_mul(out=dotv[:], in0=dotu[:], scalar1=alpha[:])
        ps = pr.tile([1, 1], f32, tag="pr")
        nc.tensor.matmul(ps[:], lhsT=ones[:], rhs=dotv[:], start=True, stop=True)
        sb1 = small.tile([1, 1], f32, tag="sb1")
        nc.vector.tensor_copy(out=sb1[:], in_=ps[:])
        nc.sync.dma_start(out=poss_out[:, :], in_=sb1[:])

        # ||p_bf16||^2, in the precision the stage-B PE will see (dis diag)
        rsqt = small.tile([P, 1], f32, tag="rsqt")
        scr3 = scrp.tile([P, D], f32, tag="scr")
        nc.scalar.activation(out=scr3[:], in_=pbf[:], func=AF.Square, accum_out=rsqt[:])
        nc.scalar.dma_start(out=rsq_out[:, :], in_=rsqt[:])

        # transpose own prototypes [classes, D] -> 4 x [128d, 128c] blocks
        for k in range(KT):
            tp = pt.tile([P, P], bf16, tag="tp", name=f"tp{k}")
            nc.tensor.transpose(tp[:], pbf[:, k * P : (k + 1) * P], ident[:])
            nc.scalar.copy(out=po[k][:], in_=tp[:])
            nc.gpsimd.dma_start(out=ptc_out[k * P : (k + 1) * P, :], in_=po[k][:])
    nc.finalize()
    return nc


def _build_stage_b():
    from contextlib import ExitStack

    import concourse.bacc as bacc
    import concourse.tile as tile
    from concourse import mybir

    f32 = mybir.dt.float32
    bf16 = mybir.dt.bfloat16
    AF = mybir.ActivationFunctionType
    OP = mybir.AluOpType
    AX = mybir.AxisListType

    nc = bacc.Bacc(None)
    featT = nc.dram_tensor("featT", [D, BPC], bf16, kind="ExternalInput")
    ptA = nc.dram_tensor("ptA", [D, CPAD], bf16, kind="ExternalInput")
    ptO = nc.dram_tensor("ptO", [D, P], bf16, kind="ExternalInput")
    rsqd = nc.dram_tensor("rsq", [P, 1], f32, kind="ExternalInput")
    res_out = nc.dram_tensor("res", [1, 2], f32, kind="ExternalOutput")

    with tile.TileContext(nc) as tc, ExitStack() as ctx:
        pers = ctx.enter_context(tc.tile_pool(name="pers", bufs=1))
        scrp = ctx.enter_context(tc.tile_pool(name="scrp", bufs=2))
        small = ctx.enter_context(tc.tile_pool(name="small", bufs=4))
        pp = ctx.enter_context(tc.tile_pool(name="pp", bufs=6, space="PSUM"))
        pr = ctx.enter_context(tc.tile_pool(name="pr", bufs=1, space="PSUM"))

        ft = [pers.tile([P, BPC], bf16, name=f"ft{k}") for k in range(KT)]
        rh = [pers.tile([P, CPAD], bf16, name=f"rh{k}") for k in range(KT)]
        po = [pers.tile([P, P], bf16, name=f"po{k}") for k in range(KT)]
        ones = pers.tile([P, 1], f32)
        rsq = small.tile([P, 1], f32, tag="rsq")
        negb_all = pers.tile([P, MT], f32)
        ses_all = pers.tile([P, MT], f32)
        rhs2 = pers.tile([P, 2], f32)
        nc.vector.memset(ones[:], 1.0)
        nc.vector.memset(rhs2[:], 0.0)

        # interleave so m=0/k=0 operands land first; two queues
        for k in range(KT):
            nc.sync.dma_start(out=rh[k][:], in_=ptA[k * P : (k + 1) * P, :])
            nc.gpsimd.dma_start(out=ft[k][:], in_=featT[k * P : (k + 1) * P, :])
        for k in range(KT):
            nc.gpsimd.dma_start(out=po[k][:], in_=ptO[k * P : (k + 1) * P, :])
        nc.sync.dma_start(out=rsq[:], in_=rsqd[:, :])

        diag = small.tile([P, 1], f32, tag="diag")
        nc.scalar.activation(out=diag[:], in_=rsq[:], func=AF.Exp, scale=10.0)

        # m = 0..7: comp logits for own batch rows; m = 8: dis rows
        for m in range(MT + 1):
            pc = [
                pp.tile([P, NH], f32, tag="pc", name=f"pc{m}_{i}") for i in range(2)
            ]
            for k in range(KT):
                lh = ft[k][:, m * P : (m + 1) * P] if m < MT else po[k][:]
                for nk in range(2):
                    nc.tensor.matmul(
                        pc[nk][:],
                        lhsT=lh,
                        rhs=rh[k][:, nk * NH : (nk + 1) * NH],
                        start=(k == 0),
                        stop=(k == KT - 1),
                    )
            if m < MT:
                m0 = small.tile([P, 1], f32, tag="m0")
                m1 = small.tile([P, 1], f32, tag="m1")
                nc.vector.reduce_max(out=m0[:], in_=pc[0][:], axis=AX.X)
                nc.vector.reduce_max(out=m1[:], in_=pc[1][:], axis=AX.X)
                nc.vector.tensor_scalar(
                    out=negb_all[:, m : m + 1], in0=m0[:], scalar1=m1[:],
                    scalar2=-10.0, op0=OP.max, op1=OP.mult,
                )
                ses01 = small.tile([P, 2], f32, tag="ses01")
                for nk in range(2):
                    e = scrp.tile([P, NH], f32, tag="escr")
                    nc.scalar.activation(
                        out=e[:], in_=pc[nk][:], func=AF.Exp,
                        bias=negb_all[:, m : m + 1], scale=10.0,
                        accum_out=ses01[:, nk : nk + 1],
                    )
                # pad columns carry z=0: subtract their exp(negb) exactly
                pe_ = small.tile([P, 1], f32, tag="pe")
                nc.scalar.activation(out=pe_[:], in_=negb_all[:, m : m + 1], func=AF.Exp)
                s01 = small.tile([P, 1], f32, tag="s01")
                nc.gpsimd.tensor_add(out=s01[:], in0=ses01[:, 0:1], in1=ses01[:, 1:2])
                t24 = small.tile([P, 1], f32, tag="t24")
                nc.vector.tensor_scalar_mul(
                    out=t24[:], in0=pe_[:], scalar1=-float(CPAD - C)
                )
                nc.vector.tensor_add(
                    out=ses_all[:, m : m + 1], in0=t24[:], in1=s01[:]
                )
            else:
                ses_d = small.tile([P, 2], f32, tag="sesd")
                for nk in range(2):
                    e = scrp.tile([P, NH], f32, tag="escr")
                    nc.scalar.activation(
                        out=e[:], in_=pc[nk][:], func=AF.Exp, scale=10.0,
                        accum_out=ses_d[:, nk : nk + 1],
                    )
                rowsum = small.tile([P, 1], f32, tag="rowsum")
                nc.vector.reduce_sum(out=rowsum[:], in_=ses_d[:], axis=AX.X)
                # masked = rowsum - diag - (#pad columns, each exp(0)=1)
                masked = small.tile([P, 1], f32, tag="masked")
                nc.vector.tensor_scalar(
                    out=masked[:], in0=rowsum[:], scalar1=diag[:],
                    scalar2=float(CPAD - C), op0=OP.subtract, op1=OP.subtract,
                )
                nc.scalar.activation(
                    out=rhs2[:CPC, 1:2], in_=masked[:CPC], func=AF.Ln
                )

        # comp tail: sum over rows of (ln(ses) - negb)
        ln_all = pers.tile([P, MT], f32)
        nc.scalar.activation(out=ln_all[:], in_=ses_all[:], func=AF.Ln)
        term = pers.tile([P, MT], f32)
        nc.vector.tensor_sub(out=term[:], in0=ln_all[:], in1=negb_all[:])
        nc.vector.reduce_sum(out=rhs2[:, 0:1], in_=term[:], axis=AX.X)

        ps = pr.tile([1, 2], f32, tag="pr")
        nc.tensor.matmul(ps[:], lhsT=ones[:], rhs=rhs2[:], start=True, stop=True)
        sb2 = small.tile([1, 2], f32, tag="sb2")
        nc.vector.tensor_copy(out=sb2[:], in_=ps[:])
        nc.sync.dma_start(out=res_out[:, :], in_=sb2[:])
    nc.finalize()
    return nc


def _get_stage_a(L1):
    key = ("A", L1)
    if key not in _CACHE:
        _CACHE[key] = _build_stage_a(L1)
    return _CACHE[key]


def _get_stage_b():
    if "B" not in _CACHE:
        _CACHE["B"] = _build_stage_b()
    return _CACHE["B"]


def kernel(features, prototypes, labels):
    import ml_dtypes

    from concourse.bass_utils import run_bass_kernel_spmd

    bf16 = ml_dtypes.bfloat16
    f32 = np.float32
    features = np.ascontiguousarray(features, dtype=f32)
    prototypes = np.ascontiguousarray(prototypes, dtype=f32)
    labels = np.asarray(labels)

    # ---- host prep: per-class chains, right-aligned last-LT window with
    # sqrt(2)^j weights (older terms have geometrically negligible weight) ----
    order = np.argsort(labels, kind="stable")
    counts = np.bincount(labels, minlength=C)
    starts = np.concatenate([[0], np.cumsum(counts)])
    sorted_feats = features[order]
    lab_sorted = labels[order]
    slot = np.arange(B) - starts[lab_sorted]
    n_of = counts[lab_sorted]
    j = slot - n_of + LT
    keep = j >= 0
    w = np.float32(2.0) ** (j.astype(f32) * f32(0.5))
    core_of = lab_sorted // CPC
    row_in_core = lab_sorted % CPC

    sf_all = np.zeros((NCORES, P, D, LT), f32)
    ks = np.where(keep)[0]
    sf_all[core_of[ks], row_in_core[ks], :, j[ks]] = (
        sorted_feats[ks] * w[ks, None]
    )
    # fold p0 in for short chains (weight sqrt(2)^(LT-n); scale-free per class)
    jp = np.clip(LT - counts, 0, LT - 1)
    wp = np.where(
        counts == 0, f32(1.0), np.float32(2.0) ** (jp.astype(f32) * f32(0.5))
    ).astype(f32)
    short = np.where(counts < LT)[0]
    sf_all[short // CPC, short % CPC, :, jp[short]] += (
        wp[short, None] * prototypes[short]
    )
    sf_all = sf_all.astype(bf16)

    # unweighted per-class feature sums (for the comp positive term)
    cum = np.cumsum(sorted_feats.astype(np.float64), axis=0)
    cum = np.concatenate([np.zeros((1, D)), cum], axis=0)
    cs = (cum[starts[1:]] - cum[starts[:-1]]).astype(f32)
    cs_all = np.zeros((NCORES, P, D), f32)
    for c in range(NCORES):
        cs_all[c, :CPC] = cs[c * CPC : (c + 1) * CPC]

    # ---- launch A ----
    ncA = _get_stage_a(LT)
    in_maps = [{"sf": sf_all[c], "cs": cs_all[c]} for c in range(NCORES)]
    resA = run_bass_kernel_spmd(ncA, in_maps, list(range(NCORES))).results

    possum = np.sum(
        np.array([resA[c]["poss"][0, 0] for c in range(NCORES)], f32), dtype=f32
    )
    # gather: protoT [D, 1024] bf16 (core-major class columns; pads are zero)
    ptA = np.concatenate([resA[c]["ptc"] for c in range(NCORES)], axis=1)
    ptA = np.ascontiguousarray(ptA)

    featT = np.ascontiguousarray(features.T).astype(bf16)

    # ---- launch B ----
    ncB = _get_stage_b()
    in_maps = [
        {
            "featT": np.ascontiguousarray(featT[:, c * BPC : (c + 1) * BPC]),
            "ptA": ptA,
            "ptO": np.ascontiguousarray(ptA[:, c * P : (c + 1) * P]),
            "rsq": resA[c]["rsq"],
        }
        for c in range(NCORES)
    ]
    resB = run_bass_kernel_spmd(ncB, in_maps, list(range(NCORES))).results

    comp_total = np.sum(
        np.array([resB[c]["res"][0, 0] for c in range(NCORES)], f32), dtype=f32
    )
    dis_total = np.sum(
        np.array([resB[c]["res"][0, 1] for c in range(NCORES)], f32), dtype=f32
    )

    mean_log_prob_pos = (f32(10.0) * possum - comp_total) / f32(B)
    loss_comp = -mean_log_prob_pos
    loss_dis = dis_total / f32(C) - np.log(f32(C - 1))
    return np.array(loss_comp + loss_dis, dtype=f32)



# revision 6
# speedup vs baseline: 2.3563x; 2.3563x over previous
"""CIDER criterion (DisLoss + CompLoss) on 8 Trainium2 NeuronCores.

Single launch per core. Host does index-driven data prep (the exact
per-sample EMA prototype scan, vectorized over classes), normalization,
fp8 quantization and layout; the device does the two large matmuls
(comp logits [1024 x 1024 x 512] per core, dis logits [128 x 1024 x 512])
plus the exp row-sums, which is >99.9% of the FLOPs.

Layout: operands are pre-scaled by 16 and quantized to fp8 e4m3 (TRN
float8e4, max 240 -- identical to ml_dtypes.float8_e4m3 in range), stored
in MatmulPerfMode.DoubleRow layout [kb, p, j, cols] where contraction row
index = kb*256 + j*128 + p. Each matmul then contracts 256 rows at 0.5
cycles/output-element -- 2x the bf16 rate.

Per core: comp logits for its 1024 batch rows over all 1024 (padded)
class columns; dis logits for its 125 own classes. exp() on the Scalar
engine (scale 10/256 folds away the fp8 pre-scaling), row-sums on
Vector/GpSimd. The device ships raw row-sums [128, 9] (8 comp m-tiles +
dis); host applies ln, the exact-diagonal correction exp(10*||p||^2_fp8)
(computed from the very fp8 values the PE sees, so the e^10 diagonal
cancels), the 24 zero-pad-column correction, and the exact positive term
sum_c <cs_c, p_c> in f64.
"""

import numpy as np

# ---- problem constants (hardcoded per the harness contract) ----
B, C, D = 8192, 1000, 512
NCORES = 8
CPC = C // NCORES  # 125 classes per core
BPC = B // NCORES  # 1024 batch rows per core
P = 128
CPAD = 1024  # padded class columns (24 zero-prototype pads)
NPAD = CPAD - C
MT = BPC // P  # 8 batch tiles per core
SCALE = 16.0  # fp8 pre-scale (power of 2: keeps values out of subnormals)
ESC = 10.0 / (SCALE * SCALE)  # exp scale on device
EPS = 1e-12

_CACHE = {}


def _build(mode):
    from contextlib import ExitStack

    import concourse.bacc as bacc
    import concourse.tile as tile
    from concourse import mybir

    f32 = mybir.dt.float32
    bf16 = mybir.dt.bfloat16
    f8 = mybir.dt.float8e4
    AF = mybir.ActivationFunctionType
    AX = mybir.AxisListType

    dr = mode == "fp8dr"
    idt = f8 if mode.startswith("fp8") else bf16
    if dr:
        KB, KS = 2, 2  # 2 blocks of (128 x 2) contraction rows
        pm = mybir.MatmulPerfMode.DoubleRow
    else:
        KB, KS = 4, 1  # 4 plain 128-row chunks
        pm = None

    nc = bacc.Bacc(None)
    ftd = nc.dram_tensor("ft", [KB, P, KS, BPC], idt, kind="ExternalInput")
    rhd = nc.dram_tensor("rh", [KB, P, KS, CPAD], idt, kind="ExternalInput")
    pod = nc.dram_tensor("po", [KB, P, KS, P], idt, kind="ExternalInput")
    res_out = nc.dram_tensor("res", [P, MT + 1], f32, kind="ExternalOutput")

    with tile.TileContext(nc) as tc, ExitStack() as ctx:
        pers = ctx.enter_context(tc.tile_pool(name="pers", bufs=1))
        scrp = ctx.enter_context(tc.tile_pool(name="scrp", bufs=3))
        pp = ctx.enter_context(tc.tile_pool(name="pp", bufs=3, space="PSUM"))

        ftt = [pers.tile([P, KS, BPC], idt, name=f"ftt{k}") for k in range(KB)]
        rht = [pers.tile([P, KS, CPAD], idt, name=f"rht{k}") for k in range(KB)]
        pot = [pers.tile([P, KS, P], idt, name=f"pot{k}") for k in range(KB)]
        ses = pers.tile([P, MT + 1], f32)

        # warm the Exp activation table while DMAs stream
        warm = pers.tile([1, 1], f32)
        nc.vector.memset(warm[:], 0.0)
        nc.scalar.activation(out=warm[:], in_=warm[:], func=AF.Exp)

        # interleave input DMAs over the three DMA-capable queues,
        # first-needed chunks first
        qs = [nc.sync, nc.scalar, nc.gpsimd]
        for k in range(KB):
            qs[k % 2].dma_start(out=rht[k][:], in_=rhd[k])
            qs[1 - k % 2].dma_start(out=ftt[k][:], in_=ftd[k])
        for k in range(KB):
            nc.gpsimd.dma_start(out=pot[k][:], in_=pod[k])

        # m = 0..7: comp logits for own batch rows; m = 8: dis rows
        for m in range(MT + 1):
            pc = pp.tile([P, CPAD], f32, tag="pc", name=f"pc{m}")
            for kb in range(KB):
                lh = (
                    ftt[kb][:, :, m * P : (m + 1) * P] if m < MT else pot[kb][:]
                )
                for nh in range(2):
                    nc.tensor.matmul(
                        pc[:, nh * 512 : (nh + 1) * 512],
                        lhsT=lh,
                        rhs=rht[kb][:, :, nh * 512 : (nh + 1) * 512],
                        start=(kb == 0),
                        stop=(kb == KB - 1),
                        perf_mode=pm,
                    )
            # dis tile (m=8) keeps f32: its row-sum carries the e^10 diagonal
            # that the host subtracts exactly; bf16 would round it by ~2e-3.
            if m < MT:
                e = scrp.tile([P, CPAD], bf16, tag="e", name=f"e{m}")
            else:
                e = scrp.tile([P, CPAD], f32, tag="ed", name="ed")
            nc.scalar.activation(out=e[:], in_=pc[:], func=AF.Exp, scale=ESC)
            nc.vector.reduce_sum(out=ses[:, m : m + 1], in_=e[:], axis=AX.X)

        nc.sync.dma_start(out=res_out[:, :], in_=ses[:])
    nc.finalize()
    return nc


def _get(mode):
    if mode not in _CACHE:
        _CACHE[mode] = _build(mode)
    return _CACHE[mode]


def _dr_layout(a):
    # [512, N] -> [2, 128, 2, N] with contraction row = kb*256 + j*128 + p
    n = a.shape[1]
    return np.ascontiguousarray(a.reshape(2, 2, P, n).transpose(0, 2, 1, 3))


def _k4_layout(a):
    n = a.shape[1]
    return np.ascontiguousarray(a.reshape(4, P, 1, n))


def kernel(features, prototypes, labels, mode="fp8dr"):
    import ml_dtypes

    from concourse.bass_utils import run_bass_kernel_spmd

    f32 = np.float32
    np_idt = ml_dtypes.float8_e4m3 if mode.startswith("fp8") else ml_dtypes.bfloat16
    features = np.ascontiguousarray(features, dtype=f32)
    prototypes = np.ascontiguousarray(prototypes, dtype=f32)
    labels = np.asarray(labels).astype(np.int64)

    # ---- exact EMA scan, vectorized across classes (order within a class
    # is batch order; classes are independent) ----
    order = np.argsort(labels, kind="stable")
    sf = features[order]
    sl = labels[order]
    counts = np.bincount(labels, minlength=C)
    starts = np.concatenate([[0], np.cumsum(counts)])[:-1]
    proto = prototypes.copy()
    for s in range(int(counts.max())):
        sel = counts > s
        idx = starts[sel] + s
        cls = sl[idx]
        upd = proto[cls] * f32(0.5) + sf[idx] * f32(0.5)
        n = np.sqrt(np.sum(upd * upd, axis=1, keepdims=True, dtype=f32))
        proto[cls] = upd / np.maximum(n, f32(EPS))

    pn = proto / np.maximum(
        np.sqrt(np.sum(proto * proto, axis=1, keepdims=True, dtype=f32)), f32(EPS)
    )

    # ---- quantize (scaled), pad classes to 1024, build device layouts ----
    pTq = np.zeros((D, CPAD), np_idt)
    pTq[:, :C] = (pn.T * f32(SCALE)).astype(np_idt)
    fTq = (features.T * f32(SCALE)).astype(np_idt)

    lay = _dr_layout if mode == "fp8dr" else _k4_layout
    rh_dr = lay(pTq)
    ft_dr = lay(fTq)

    pq32 = pTq.astype(f32)
    rsq_scaled = np.sum(pq32 * pq32, axis=0, dtype=f32)  # [1024]

    # exact positive term in f64 (unquantized prototypes)
    cs = np.zeros((C, D), np.float64)
    np.add.at(cs, labels, features.astype(np.float64))
    possum = float(np.sum(cs * pn.astype(np.float64)))

    in_maps = []
    for c in range(NCORES):
        c0 = c * CPC
        in_maps.append(
            {
                "ft": np.ascontiguousarray(
                    ft_dr[:, :, :, c * BPC : (c + 1) * BPC]
                ),
                "rh": rh_dr,
                "po": np.ascontiguousarray(rh_dr[:, :, :, c0 : c0 + P]),
            }
        )

    ncb = _get(mode)
    res = run_bass_kernel_spmd(ncb, in_maps, list(range(NCORES))).results

    # ---- host combine (f64; ln of 8192 + 1000 row-sums) ----
    comp_total = 0.0
    dis_total = 0.0
    for c in range(NCORES):
        r = res[c]["res"].astype(np.float64)  # [128, 9]
        comp_total += np.sum(np.log(r[:, :MT] - NPAD))
        c0 = c * CPC
        diag = np.exp(rsq_scaled[c0 : c0 + CPC].astype(np.float64) * ESC)
        dis_total += np.sum(np.log(r[:CPC, MT] - diag - NPAD))

    mean_log_prob_pos = (10.0 * possum - comp_total) / B
    loss_comp = -mean_log_prob_pos
    loss_dis = dis_total / C - np.log(float(C - 1))
    return np.array(loss_comp + loss_dis, dtype=f32)


# revision 7
# speedup vs baseline: 2.5863x; 1.0976x over previous
"""CIDER criterion (DisLoss + CompLoss) on 8 Trainium2 NeuronCores.

Single launch per core. Host does index-driven data prep (the exact
per-sample EMA prototype scan, vectorized over classes), normalization,
fp8 quantization and layout; the device does the two large matmuls
(comp logits [1024 x 1024 x 512] per core, dis logits [128 x 1024 x 512])
plus the exp row-sums, which is >99.9% of the FLOPs.

Key design points:
- Operands pre-scaled by 16, quantized to fp8 e4m3 (TRN float8e4), in
  MatmulPerfMode.DoubleRow layout: SBUF tile [p, kb, j, col] contracts
  row kb*256 + j*128 + p. 0.5 cycles/output element = 2x bf16 rate.
- protoT columns are rolled per core so its own 125 dis classes always
  sit at columns 0..124 (comp row-sums are permutation invariant), so
  one static program serves all 8 cores with no per-core proto input.
- One dma_start per operand (128 descriptors x 4KB each, HWDGE) to
  minimize descriptor-issue latency; PE warm-up dummies run during the
  DMA window so the real matmuls start at full clock.
- Device ships raw exp row-sums [16, 128] (PE-transposed, 16 big
  descriptors); host applies ln, the exact e^10 diagonal correction
  (from the same fp8 values the PE sees), the 24-pad-column correction,
  and the exact positive term sum_c <cs_c, p_c> in f64.
"""

import numpy as np

# ---- problem constants (hardcoded per the harness contract) ----
B, C, D = 8192, 1000, 512
NCORES = 8
CPC = C // NCORES  # 125 classes per core
BPC = B // NCORES  # 1024 batch rows per core
P = 128
CPAD = 1024  # padded class columns (24 zero-prototype pads)
NPAD = CPAD - C
MT = BPC // P  # 8 batch tiles per core
KB = 2  # two (128 x 2)-row DoubleRow contraction blocks
SCALE = 16.0  # fp8 pre-scale (power of 2: keeps values out of subnormals)
ESC = 10.0 / (SCALE * SCALE)  # exp scale on device
EPS = 1e-12
WARM_MM = 10  # PE warm-up dummy matmuls issued during the DMA window

_CACHE = {}


def _build():
    from contextlib import ExitStack

    import concourse.bacc as bacc
    import concourse.tile as tile
    from concourse import masks, mybir

    f32 = mybir.dt.float32
    bf16 = mybir.dt.bfloat16
    f8 = mybir.dt.float8e4
    AF = mybir.ActivationFunctionType
    AX = mybir.AxisListType
    DR = mybir.MatmulPerfMode.DoubleRow

    nc = bacc.Bacc(None)
    ftd = nc.dram_tensor("ft", [P, KB, 2, BPC], f8, kind="ExternalInput")
    rhd = nc.dram_tensor("rh", [P, KB, 2, CPAD], f8, kind="ExternalInput")
    res_out = nc.dram_tensor("res", [16, P], f32, kind="ExternalOutput")

    with tile.TileContext(nc) as tc, ExitStack() as ctx:
        pers = ctx.enter_context(tc.tile_pool(name="pers", bufs=1))
        scrp = ctx.enter_context(tc.tile_pool(name="scrp", bufs=3))
        pp = ctx.enter_context(tc.tile_pool(name="pp", bufs=3, space="PSUM"))
        pw = ctx.enter_context(tc.tile_pool(name="pw", bufs=1, space="PSUM"))
        pr = ctx.enter_context(tc.tile_pool(name="pr", bufs=1, space="PSUM"))

        ftt = pers.tile([P, KB, 2, BPC], f8)
        rht = pers.tile([P, KB, 2, CPAD], f8)
        ses = pers.tile([P, 16], f32)
        ident = pers.tile([P, P], f32)

        # warm the Exp activation table while DMAs stream
        warm = pers.tile([1, 1], f32)
        nc.vector.memset(warm[:], 0.0)
        nc.scalar.activation(out=warm[:], in_=warm[:], func=AF.Exp)

        masks.make_identity(nc, ident[:])

        # one big dma_start per operand: 128 contiguous 4KB descriptors
        nc.sync.dma_start(out=rht[:], in_=rhd[:, :, :, :])
        nc.scalar.dma_start(out=ftt[:], in_=ftd[:, :, :, :])

        # PE warm-up: keep the PE busy during the DMA window so HAM has
        # ramped the clock by the time the real matmuls arrive
        wps = pw.tile([P, P], f32, tag="wps")
        for _ in range(WARM_MM):
            nc.tensor.matmul(
                wps[:], lhsT=ident[:], rhs=ident[:], start=True, stop=True
            )

        # m = 0..7: comp logits for own batch rows; m = 8: dis rows
        # (own classes are columns 0..127 of the rolled protoT)
        for m in range(MT + 1):
            pc = pp.tile([P, CPAD], f32, tag="pc", name=f"pc{m}")
            for kb in range(KB):
                lh = (
                    ftt[:, kb, :, m * P : (m + 1) * P]
                    if m < MT
                    else rht[:, kb, :, 0:P]
                )
                for nh in range(2):
                    nc.tensor.matmul(
                        pc[:, nh * 512 : (nh + 1) * 512],
                        lhsT=lh,
                        rhs=rht[:, kb, :, nh * 512 : (nh + 1) * 512],
                        start=(kb == 0),
                        stop=(kb == KB - 1),
                        perf_mode=DR,
                    )
            # dis tile (m=8) keeps f32: its row-sum carries the e^10 diagonal
            # that the host subtracts exactly; bf16 would round it by ~2e-3.
            if m < MT:
                e = scrp.tile([P, CPAD], bf16, tag="e", name=f"e{m}")
            else:
                e = scrp.tile([P, CPAD], f32, tag="ed", name="ed")
            nc.scalar.activation(out=e[:], in_=pc[:], func=AF.Exp, scale=ESC)
            nc.vector.reduce_sum(out=ses[:, m : m + 1], in_=e[:], axis=AX.X)

        # transpose row-sums [128, 16] -> [16, 128] so the output DMA is
        # 16 big descriptors instead of 128 tiny ones
        pt = pr.tile([16, P], f32, tag="pt")
        nc.tensor.transpose(pt[:], ses[:], ident[:])
        sb = pers.tile([16, P], f32)
        nc.vector.tensor_copy(out=sb[:], in_=pt[:])
        nc.sync.dma_start(out=res_out[:, :], in_=sb[:])
    nc.finalize()
    return nc


def _get():
    if "nc" not in _CACHE:
        _CACHE["nc"] = _build()
    return _CACHE["nc"]


def _dr_layout(a):
    # [512, N] -> [128, 2, 2, N] with contraction row = kb*256 + j*128 + p
    n = a.shape[1]
    return np.ascontiguousarray(a.reshape(KB, 2, P, n).transpose(2, 0, 1, 3))


def kernel(features, prototypes, labels):
    import ml_dtypes

    from concourse.bass_utils import run_bass_kernel_spmd

    f32 = np.float32
    f8 = ml_dtypes.float8_e4m3
    features = np.ascontiguousarray(features, dtype=f32)
    prototypes = np.ascontiguousarray(prototypes, dtype=f32)
    labels = np.asarray(labels).astype(np.int64)

    # ---- exact EMA scan, vectorized across classes (order within a class
    # is batch order; classes are independent) ----
    order = np.argsort(labels, kind="stable")
    sf = features[order]
    sl = labels[order]
    counts = np.bincount(labels, minlength=C)
    starts = np.concatenate([[0], np.cumsum(counts)])[:-1]
    proto = prototypes.copy()
    for s in range(int(counts.max())):
        sel = counts > s
        idx = starts[sel] + s
        cls = sl[idx]
        upd = proto[cls] * f32(0.5) + sf[idx] * f32(0.5)
        n = np.sqrt(np.sum(upd * upd, axis=1, keepdims=True, dtype=f32))
        proto[cls] = upd / np.maximum(n, f32(EPS))

    pn = proto / np.maximum(
        np.sqrt(np.sum(proto * proto, axis=1, keepdims=True, dtype=f32)), f32(EPS)
    )

    # ---- quantize (scaled), pad classes to 1024, build device layouts ----
    pTq = np.zeros((D, CPAD), f8)
    pTq[:, :C] = (pn.T * f32(SCALE)).astype(f8)
    fTq = (features.T * f32(SCALE)).astype(f8)
    ft_dr = _dr_layout(fTq)  # [128, 2, 2, 8192]

    pq32 = pTq.astype(f32)
    rsq_scaled = np.sum(pq32 * pq32, axis=0, dtype=f32)  # [1024]

    # exact positive term in f64 (unquantized prototypes)
    cs = np.zeros((C, D), np.float64)
    np.add.at(cs, labels, features.astype(np.float64))
    possum = float(np.sum(cs * pn.astype(np.float64)))

    in_maps = []
    for c in range(NCORES):
        # roll so core c's own classes are columns 0..124
        rolled = np.roll(pTq, -c * CPC, axis=1)
        in_maps.append(
            {
                "ft": np.ascontiguousarray(
                    ft_dr[:, :, :, c * BPC : (c + 1) * BPC]
                ),
                "rh": _dr_layout(rolled),
            }
        )

    ncb = _get()
    res = run_bass_kernel_spmd(ncb, in_maps, list(range(NCORES))).results

    # ---- host combine (f64; ln of 8192 + 1000 row-sums) ----
    comp_total = 0.0
    dis_total = 0.0
    for c in range(NCORES):
        r = res[c]["res"].astype(np.float64)  # [16, 128]
        comp_total += np.sum(np.log(r[:MT, :] - NPAD))
        c0 = c * CPC
        diag = np.exp(rsq_scaled[c0 : c0 + CPC].astype(np.float64) * ESC)
        dis_total += np.sum(np.log(r[MT, :CPC] - diag - NPAD))

    mean_log_prob_pos = (10.0 * possum - comp_total) / B
    loss_comp = -mean_log_prob_pos
    loss_dis = dis_total / C - np.log(float(C - 1))
    return np.array(loss_comp + loss_dis, dtype=f32)
